# revision 1
# baseline (speedup 1.0000x reference)
"""Trainium2 kernel for nn_AxialAttentionBlockAISummer.

Strategy: data-parallel over the batch axis across the 8 NeuronCores
(one image per core).  All BatchNorm statistics are computed globally —
each core computes its local (mean, mean-of-squares) partial moments and
they are combined with cross-core pmean collectives, so the result
matches the single-device reference exactly (same math, same reduction
sizes).  Within a core the axial attention is batched over the (b*w) /
(b*h) rows.  Weights are replicated.
"""

import numpy as np

B, C_IN, DIM = 8, 256, 64
HEADS, D_IN, DKQ = 8, 128, 8
DV = D_IN // HEADS            # 16
QKV = 2 * DKQ + DV            # 32
EPS = 1e-5
N_CORES = 8

_compiled = None


def _build():
    import jax
    import jax.numpy as jnp
    from jax.sharding import Mesh, PartitionSpec as P
    try:
        from jax.experimental.shard_map import shard_map
    except ImportError:
        from jax.sharding import shard_map  # newer jax

    devs = jax.devices()[:N_CORES]
    mesh = Mesh(np.asarray(devs), ("b",))

    def _bn(x, gamma, beta, ch_axis=1):
        # global batch statistics: local moments + cross-core mean
        axes = tuple(i for i in range(x.ndim) if i != ch_axis)
        m1 = jnp.mean(x, axes, keepdims=True)
        m2 = jnp.mean(x * x, axes, keepdims=True)
        m1 = jax.lax.pmean(m1, "b")
        m2 = jax.lax.pmean(m2, "b")
        var = m2 - m1 * m1
        shp = [1] * x.ndim
        shp[ch_axis] = -1
        return (x - m1) * jax.lax.rsqrt(var + EPS) * gamma.reshape(shp) \
            + beta.reshape(shp)

    def _rel_qkv(rel):
        idx = jnp.arange(DIM)[:, None] - jnp.arange(DIM)[None, :] + DIM - 1
        emb = rel[:, idx.reshape(-1)].reshape(QKV, DIM, DIM)
        return emb[:DKQ], emb[DKQ:2 * DKQ], emb[2 * DKQ:]

    def _axial_att(x, w_qkv, rel, ga, ba, go, bo):
        b = x.shape[0]
        qkv = jnp.einsum("oc,bcd->bod", w_qkv, x)
        qkv = qkv.reshape(b, QKV, HEADS, DIM).transpose(0, 2, 1, 3)
        q = qkv[:, :, :DKQ]
        k = qkv[:, :, DKQ:2 * DKQ]
        v = qkv[:, :, 2 * DKQ:]
        r_q, r_k, r_v = _rel_qkv(rel)
        qr = jnp.einsum("bhid,idj->bhdj", q, r_q)
        kr = jnp.einsum("bhid,idj->bhdj", k, r_k)
        dots = jnp.einsum("bhid,bhij->bhdj", q, k)
        cat = jnp.stack([qr, kr, dots], 0).transpose(1, 2, 0, 3, 4)
        cat = cat.reshape(b, HEADS * 3, DIM, DIM)
        cat = _bn(cat, ga, ba)
        logits = cat.reshape(b, HEADS, 3, DIM, DIM).sum(axis=2)
        attn = jax.nn.softmax(logits, axis=-1)
        sv = jnp.einsum("bhdj,bhij->bhid", attn, v)
        sve = jnp.einsum("bhdj,idj->bhid", attn, r_v)
        out = jnp.stack([sve, sv], 0).transpose(1, 0, 2, 3, 4)
        out = out.reshape(b, 2 * D_IN, DIM)
        out = _bn(out, go, bo)
        return out.reshape(b, 2, D_IN, DIM).sum(axis=1)

    def fwd(x_in, w_in, g_in, b_in, w_out, g_out, b_out,
            wqkv_h, rel_h, ga_h, ba_h, go_h, bo_h,
            wqkv_w, rel_w, ga_w, ba_w, go_w, bo_w):
        bl = x_in.shape[0]  # local batch (B / N_CORES)
        x = jax.nn.relu(_bn(jnp.einsum("oc,bchw->bohw", w_in, x_in),
                            g_in, b_in))
        x = x.transpose(0, 3, 1, 2).reshape(bl * DIM, D_IN, DIM)
        x = _axial_att(x, wqkv_h, rel_h, ga_h, ba_h, go_h, bo_h)
        x = x.reshape(bl, DIM, D_IN, DIM).transpose(0, 3, 2, 1)
        x = x.reshape(bl * DIM, D_IN, DIM)
        x = jax.nn.relu(_axial_att(x, wqkv_w, rel_w, ga_w, ba_w, go_w, bo_w))
        x = x.reshape(bl, DIM, D_IN, DIM).transpose(0, 2, 1, 3)
        y = _bn(jnp.einsum("oc,bchw->bohw", w_out, x), g_out, b_out) + x_in
        return jax.nn.relu(y)

    arg_order = ["x_in", "w_in", "g_in", "b_in", "w_out", "g_out", "b_out",
                 "wqkv_h", "rel_h", "ga_h", "ba_h", "go_h", "bo_h",
                 "wqkv_w", "rel_w", "ga_w", "ba_w", "go_w", "bo_w"]
    in_specs = tuple(P("b") if n == "x_in" else P() for n in arg_order)
    fn = jax.jit(shard_map(fwd, mesh=mesh, in_specs=in_specs,
                           out_specs=P("b"), check_rep=False))
    return fn, arg_order


def kernel(**inputs):
    global _compiled
    if _compiled is None:
        _compiled = _build()
    fn, arg_order = _compiled
    args = [np.asarray(inputs[n], np.float32) for n in arg_order]
    out = fn(*args)
    return np.asarray(out, np.float32)


if __name__ == "__main__":
    rng = np.random.default_rng(0)
    ins = {
        "x_in": rng.standard_normal((B, C_IN, DIM, DIM), np.float32),
        "w_in": (rng.standard_normal((D_IN, C_IN)) * 0.05).astype(np.float32),
        "g_in": np.ones(D_IN, np.float32),
        "b_in": np.zeros(D_IN, np.float32),
        "w_out": (rng.standard_normal((C_IN, D_IN)) * 0.05).astype(np.float32),
        "g_out": np.ones(C_IN, np.float32),
        "b_out": np.zeros(C_IN, np.float32),
    }
    for tag in ("h", "w"):
        ins["wqkv_" + tag] = (rng.standard_normal((HEADS * QKV, D_IN)) * 0.05
                              ).astype(np.float32)
        ins["rel_" + tag] = (rng.standard_normal((QKV, 2 * DIM - 1)) * 0.1
                             ).astype(np.float32)
        ins["ga_" + tag] = np.ones(HEADS * 3, np.float32)
        ins["ba_" + tag] = np.zeros(HEADS * 3, np.float32)
        ins["go_" + tag] = np.ones(2 * D_IN, np.float32)
        ins["bo_" + tag] = np.zeros(2 * D_IN, np.float32)
    y = kernel(**ins)
    print("kernel out", y.shape, y.dtype, float(np.abs(y).max()))
